# revision 5
# baseline (speedup 1.0000x reference)
"""Trainium2 Bass kernel for nn_BasicResidualBlock (spiking CNN block).

Computation (per reference):
    s1 = IF_scan(x)                 # v += x; s = H(v-1); v *= (1-s)
    y1 = conv3x3(s1, w1) * inv1 + shift1
    s2 = IF_scan(y1)
    out = conv3x3(s2, w2) * inv2 + shift2

Shapes: x [T=8, B=32, C=128, H=32, W=32] fp32.

Strategy:
  - Data-parallel over B across 8 cores (4 images per core).
  - Per (t, b) image: channels C=128 on SBUF partitions, H*W on the free dim.
  - IF neuron state v kept in a zero-padded [128, 34*34] layout so the 3x3
    conv taps can read shifted windows directly (pad border stays exactly 0
    through the IF ops: is_ge(0,1)=0, (0 is_lt 1)*0 = 0).
  - conv3x3 = 9 shifted matmuls accumulating in PSUM; spikes are exactly
    representable in bf16, so products w*s are exact up to the weight
    rounding. BN scale is folded into the weights host-side; BN shift is a
    per-channel bias applied by the scalar engine on the PSUM->SBUF copy.
  - conv1 uses a 2-way bf16 hi/lo weight split (~fp32-precision weights;
    needed because the downstream spike threshold amplifies weight rounding
    into spike flips); conv2 feeds the output directly and uses 1 split.
  - Software-pipelined: conv2/output of image i-1 is emitted between conv1
    of image i and i+1 so the PE never waits on the vector-engine IF ops.
"""

import sys

import numpy as np

if "/opt/trn_rl_repo" not in sys.path:
    sys.path.insert(0, "/opt/trn_rl_repo")

import ml_dtypes

EPS = 1e-5
N_CORES = 8
T, B, C, H, W = 8, 32, 128, 32, 32
NB = B // N_CORES          # images per core per timestep
N_IMGS = T * NB            # images per core
HW = H * W                 # 1024
PW = W + 2                 # padded row width 34
PHW = (H + 2) * PW         # 1156
CONV1_SPLITS = 2
CONV2_SPLITS = 1

_program_cache = {}


def build_program(n_imgs=N_IMGS, n_b=NB, conv1_splits=CONV1_SPLITS,
                  conv2_splits=CONV2_SPLITS):
    import concourse.mybir as mybir
    from concourse.bacc import Bacc
    from concourse.tile import TileContext

    f32 = mybir.dt.float32
    bf16 = mybir.dt.bfloat16
    Alu = mybir.AluOpType
    Act = mybir.ActivationFunctionType

    nc = Bacc()
    x_d = nc.declare_dram_parameter("x", [n_imgs, C, HW], f32, isOutput=False)
    w1_d = nc.declare_dram_parameter("w1", [C, 9 * conv1_splits * C], bf16,
                                     isOutput=False)
    w2_d = nc.declare_dram_parameter("w2", [C, 9 * conv2_splits * C], bf16,
                                     isOutput=False)
    b1_d = nc.declare_dram_parameter("b1", [C, 1], f32, isOutput=False)
    b2_d = nc.declare_dram_parameter("b2", [C, 1], f32, isOutput=False)
    y_d = nc.declare_dram_parameter("y", [n_imgs, C, HW], f32, isOutput=True)

    with TileContext(nc) as tc:
        with (
            tc.tile_pool(name="const", bufs=1) as cp,
            tc.tile_pool(name="state", bufs=1) as vp,
            tc.tile_pool(name="work", bufs=2) as wp,
            tc.tile_pool(name="psum", bufs=4, space="PSUM") as pp,
        ):
            w1s = cp.tile([C, 9 * conv1_splits * C], bf16, tag="w1s", name="w1s")
            w2s = cp.tile([C, 9 * conv2_splits * C], bf16, tag="w2s", name="w2s")
            b1s = cp.tile([C, 1], f32, tag="b1s", name="b1s")
            b2s = cp.tile([C, 1], f32, tag="b2s", name="b2s")
            nc.sync.dma_start(out=w1s, in_=w1_d[:, :])
            nc.sync.dma_start(out=w2s, in_=w2_d[:, :])
            nc.sync.dma_start(out=b1s, in_=b1_d[:, :])
            nc.sync.dma_start(out=b2s, in_=b2_d[:, :])

            v1 = [vp.tile([C, PHW], f32, tag=f"v1_{b}", name=f"v1_{b}")
                  for b in range(n_b)]
            v2 = [vp.tile([C, PHW], f32, tag=f"v2_{b}", name=f"v2_{b}")
                  for b in range(n_b)]
            for v in v1:
                nc.vector.memset(v, 0.0)
            for v in v2:
                nc.gpsimd.memset(v, 0.0)

            def if_stage(v, src, s_tile):
                # v: padded state [C, PHW]; src: [C, HW]; s_tile: [C, PHW] bf16
                vv = v.rearrange("p (h w) -> p h w", w=PW)
                sv = src.rearrange("p (h w) -> p h w", w=W)
                nc.vector.tensor_tensor(
                    out=vv[:, 1:H + 1, 1:W + 1], in0=vv[:, 1:H + 1, 1:W + 1],
                    in1=sv, op=Alu.add)
                nc.vector.tensor_scalar(
                    out=s_tile, in0=v, scalar1=1.0, scalar2=None, op0=Alu.is_ge)
                nc.vector.scalar_tensor_tensor(
                    out=v, in0=v, scalar=1.0, in1=v, op0=Alu.is_lt, op1=Alu.mult)

            def conv(s_tile, w_sb, n_splits, psum_tag):
                sv = s_tile.rearrange("p (h w) -> p h w", w=PW)
                halves = []
                nmm = 9 * n_splits
                for h2 in range(2):
                    ps = pp.tile([C, HW // 2], f32, tag=psum_tag, bufs=4,
                                 name=f"{psum_tag}_{h2}")
                    k = 0
                    for ky in range(3):
                        for kx in range(3):
                            for s_ in range(n_splits):
                                col = ((ky * 3 + kx) * n_splits + s_) * C
                                r0 = (H // 2) * h2 + ky
                                nc.tensor.matmul(
                                    out=ps,
                                    lhsT=w_sb[:, col:col + C],
                                    rhs=sv[:, r0:r0 + H // 2, kx:kx + W],
                                    start=(k == 0), stop=(k == nmm - 1))
                                k += 1
                    halves.append(ps)
                return halves

            pending = {}
            for i in range(n_imgs + 1):
                if i < n_imgs:
                    b = i % n_b
                    xt = wp.tile([C, HW], f32, tag="xt", bufs=3, name=f"xt_{i}")
                    nc.sync.dma_start(out=xt, in_=x_d[i])
                    s1 = wp.tile([C, PHW], bf16, tag="s1", bufs=3, name=f"s1_{i}")
                    if_stage(v1[b], xt, s1)
                    pending[i] = conv(s1, w1s, conv1_splits, "ps1")
                if i >= 1:
                    j = i - 1
                    b = j % n_b
                    ps1 = pending.pop(j)
                    y1 = wp.tile([C, HW], f32, tag="y1", bufs=2, name=f"y1_{j}")
                    for h2 in range(2):
                        nc.scalar.activation(
                            out=y1[:, h2 * (HW // 2):(h2 + 1) * (HW // 2)],
                            in_=ps1[h2], func=Act.Identity, bias=b1s[:, 0:1],
                            scale=1.0)
                    s2 = wp.tile([C, PHW], bf16, tag="s2", bufs=3, name=f"s2_{j}")
                    if_stage(v2[b], y1, s2)
                    ps2 = conv(s2, w2s, conv2_splits, "ps2")
                    ot = wp.tile([C, HW], f32, tag="ot", bufs=3, name=f"ot_{j}")
                    for h2 in range(2):
                        nc.scalar.activation(
                            out=ot[:, h2 * (HW // 2):(h2 + 1) * (HW // 2)],
                            in_=ps2[h2], func=Act.Identity, bias=b2s[:, 0:1],
                            scale=1.0)
                    nc.sync.dma_start(out=y_d[j], in_=ot)

    nc.finalize()
    return nc


def _split_weights(wf, n_splits):
    # wf: [O, I, 3, 3] float64 (BN scale already folded)
    lhsT = np.transpose(wf, (2, 3, 1, 0)).reshape(9, C, C)  # [tap, ci, co]
    comps, rem = [], lhsT.copy()
    for _ in range(n_splits):
        c = rem.astype(ml_dtypes.bfloat16)
        comps.append(np.asarray(c))
        rem = rem - c.astype(np.float64)
    # [tap, split, ci, co] -> [ci, tap, split, co] -> [ci, tap*split*co]
    a = np.stack(comps, axis=1)
    return np.ascontiguousarray(
        a.transpose(2, 0, 1, 3).reshape(C, 9 * n_splits * C))


def _prep(w, g, b, m, v, n_splits):
    inv = g.astype(np.float64) / np.sqrt(v.astype(np.float64) + EPS)
    wf = w.astype(np.float64) * inv[:, None, None, None]
    shift = (b.astype(np.float64) - m.astype(np.float64) * inv)
    return (_split_weights(wf, n_splits),
            shift.astype(np.float32).reshape(C, 1))


last_results = None  # BassKernelResults of the most recent run (for test.py)


def kernel(x, w1, g1, b1, m1, v1, w2, g2, b2, m2, v2, _trace=False):
    global last_results
    from concourse.bass_utils import run_bass_kernel_spmd

    x = np.asarray(x)
    assert x.shape == (T, B, C, H, W), x.shape

    key = (CONV1_SPLITS, CONV2_SPLITS)
    if key not in _program_cache:
        _program_cache[key] = build_program(
            conv1_splits=CONV1_SPLITS, conv2_splits=CONV2_SPLITS)
    nc = _program_cache[key]

    w1p, sh1 = _prep(np.asarray(w1), np.asarray(g1), np.asarray(b1),
                     np.asarray(m1), np.asarray(v1), CONV1_SPLITS)
    w2p, sh2 = _prep(np.asarray(w2), np.asarray(g2), np.asarray(b2),
                     np.asarray(m2), np.asarray(v2), CONV2_SPLITS)

    in_maps = []
    for c in range(N_CORES):
        xs = np.ascontiguousarray(
            x[:, c * NB:(c + 1) * NB].reshape(N_IMGS, C, HW))
        in_maps.append({"x": xs, "w1": w1p, "w2": w2p, "b1": sh1, "b2": sh2})

    last_results = run_bass_kernel_spmd(
        nc, in_maps, list(range(N_CORES)), trace=_trace)
    res = last_results.results
    out = np.empty((T, B, C, H, W), np.float32)
    for c in range(N_CORES):
        out[:, c * NB:(c + 1) * NB] = res[c]["y"].reshape(T, NB, C, H, W)
    return out


# revision 7
# speedup vs baseline: 1.0087x; 1.0087x over previous
"""Trainium2 Bass kernel for nn_BasicResidualBlock (spiking CNN block).

Computation (per reference):
    s1 = IF_scan(x)                 # v += x; s = H(v-1); v *= (1-s)
    y1 = conv3x3(s1, w1) * inv1 + shift1
    s2 = IF_scan(y1)
    out = conv3x3(s2, w2) * inv2 + shift2

Shapes: x [T=8, B=32, C=128, H=32, W=32] fp32.

Strategy:
  - Data-parallel over B across 8 cores (4 images per core).
  - Per (t, b) image: channels C=128 on SBUF partitions, H*W on the free dim.
  - IF neuron state v kept in a zero-padded [128, 34*34] layout so the 3x3
    conv taps can read shifted windows directly (pad border stays exactly 0
    through the IF ops: is_ge(0,1)=0, (0 is_lt 1)*0 = 0).
  - conv3x3 = 9 shifted matmuls accumulating in PSUM; spikes are exactly
    representable in bf16, so products w*s are exact up to the weight
    rounding. BN scale is folded into the weights host-side; BN shift is a
    per-channel bias applied by the scalar engine on the PSUM->SBUF copy.
  - conv1 uses a 2-way bf16 hi/lo weight split (~fp32-precision weights;
    needed because the downstream spike threshold amplifies weight rounding
    into spike flips); conv2 feeds the output directly and uses 1 split.
  - Software-pipelined: conv2/output of image i-1 is emitted between conv1
    of image i and i+1 so the PE never waits on the vector-engine IF ops.
"""

import sys

import numpy as np

if "/opt/trn_rl_repo" not in sys.path:
    sys.path.insert(0, "/opt/trn_rl_repo")

import ml_dtypes

EPS = 1e-5
N_CORES = 8
T, B, C, H, W = 8, 32, 128, 32, 32
NB = B // N_CORES          # images per core per timestep
N_IMGS = T * NB            # images per core
HW = H * W                 # 1024
PW = W + 2                 # padded row width 34
PHW = (H + 2) * PW         # 1156
CONV1_SPLITS = 2
CONV2_SPLITS = 1

_program_cache = {}


def build_program(n_imgs=N_IMGS, n_b=NB, conv1_splits=CONV1_SPLITS,
                  conv2_splits=CONV2_SPLITS):
    import concourse.mybir as mybir
    from concourse.bacc import Bacc
    from concourse.tile import TileContext

    f32 = mybir.dt.float32
    bf16 = mybir.dt.bfloat16
    Alu = mybir.AluOpType
    Act = mybir.ActivationFunctionType

    nc = Bacc()
    x_d = nc.declare_dram_parameter("x", [n_imgs, C, HW], f32, isOutput=False)
    w1_d = nc.declare_dram_parameter("w1", [C, 9 * conv1_splits * C], bf16,
                                     isOutput=False)
    w2_d = nc.declare_dram_parameter("w2", [C, 9 * conv2_splits * C], bf16,
                                     isOutput=False)
    b1_d = nc.declare_dram_parameter("b1", [C, 1], f32, isOutput=False)
    b2_d = nc.declare_dram_parameter("b2", [C, 1], f32, isOutput=False)
    y_d = nc.declare_dram_parameter("y", [n_imgs, C, HW], f32, isOutput=True)

    with TileContext(nc) as tc:
        with (
            tc.tile_pool(name="const", bufs=1) as cp,
            tc.tile_pool(name="state", bufs=1) as vp,
            tc.tile_pool(name="work", bufs=2) as wp,
            tc.tile_pool(name="psum", bufs=4, space="PSUM") as pp,
        ):
            # First input image before the (larger) weight blobs so the
            # startup-critical path (x0 -> IF -> first matmul) isn't queued
            # behind them; memsets go to the otherwise-idle GpSimd engine.
            x0 = wp.tile([C, HW], f32, tag="xt", bufs=3, name="xt_0")
            nc.sync.dma_start(out=x0, in_=x_d[0])
            w1s = cp.tile([C, 9 * conv1_splits * C], bf16, tag="w1s", name="w1s")
            w2s = cp.tile([C, 9 * conv2_splits * C], bf16, tag="w2s", name="w2s")
            b1s = cp.tile([C, 1], f32, tag="b1s", name="b1s")
            b2s = cp.tile([C, 1], f32, tag="b2s", name="b2s")
            nc.sync.dma_start(out=w1s, in_=w1_d[:, :])
            nc.sync.dma_start(out=b1s, in_=b1_d[:, :])
            nc.sync.dma_start(out=w2s, in_=w2_d[:, :])
            nc.sync.dma_start(out=b2s, in_=b2_d[:, :])

            v1 = [vp.tile([C, PHW], f32, tag=f"v1_{b}", name=f"v1_{b}")
                  for b in range(n_b)]
            v2 = [vp.tile([C, PHW], f32, tag=f"v2_{b}", name=f"v2_{b}")
                  for b in range(n_b)]
            for b in range(n_b):
                nc.gpsimd.memset(v1[b], 0.0)
            for b in range(n_b):
                nc.gpsimd.memset(v2[b], 0.0)

            def if_stage(v, src, s_tile):
                # v: padded state [C, PHW]; src: [C, HW]; s_tile: [C, PHW] bf16
                vv = v.rearrange("p (h w) -> p h w", w=PW)
                sv = src.rearrange("p (h w) -> p h w", w=W)
                nc.vector.tensor_tensor(
                    out=vv[:, 1:H + 1, 1:W + 1], in0=vv[:, 1:H + 1, 1:W + 1],
                    in1=sv, op=Alu.add)
                nc.vector.tensor_scalar(
                    out=s_tile, in0=v, scalar1=1.0, scalar2=None, op0=Alu.is_ge)
                nc.vector.scalar_tensor_tensor(
                    out=v, in0=v, scalar=1.0, in1=v, op0=Alu.is_lt, op1=Alu.mult)

            def conv(s_tile, w_sb, n_splits, psum_tag):
                sv = s_tile.rearrange("p (h w) -> p h w", w=PW)
                halves = []
                nmm = 9 * n_splits
                for h2 in range(2):
                    ps = pp.tile([C, HW // 2], f32, tag=psum_tag, bufs=4,
                                 name=f"{psum_tag}_{h2}")
                    k = 0
                    for ky in range(3):
                        for kx in range(3):
                            for s_ in range(n_splits):
                                col = ((ky * 3 + kx) * n_splits + s_) * C
                                r0 = (H // 2) * h2 + ky
                                nc.tensor.matmul(
                                    out=ps,
                                    lhsT=w_sb[:, col:col + C],
                                    rhs=sv[:, r0:r0 + H // 2, kx:kx + W],
                                    start=(k == 0), stop=(k == nmm - 1))
                                k += 1
                    halves.append(ps)
                return halves

            pending = {}
            for i in range(n_imgs + 1):
                if i < n_imgs:
                    b = i % n_b
                    if i == 0:
                        xt = x0
                    else:
                        xt = wp.tile([C, HW], f32, tag="xt", bufs=3,
                                     name=f"xt_{i}")
                        nc.sync.dma_start(out=xt, in_=x_d[i])
                    s1 = wp.tile([C, PHW], bf16, tag="s1", bufs=3, name=f"s1_{i}")
                    if_stage(v1[b], xt, s1)
                    pending[i] = conv(s1, w1s, conv1_splits, "ps1")
                if i >= 1:
                    j = i - 1
                    b = j % n_b
                    ps1 = pending.pop(j)
                    # v2 += conv1_out + shift1, straight from PSUM (one DVE op
                    # per half; no intermediate SBUF copy needed)
                    v2v = v2[b].rearrange("p (h w) -> p h w", w=PW)
                    for h2 in range(2):
                        vint = v2v[:, 1 + (H // 2) * h2:1 + (H // 2) * (h2 + 1),
                                   1:W + 1]
                        nc.vector.scalar_tensor_tensor(
                            out=vint, in0=ps1[h2].rearrange(
                                "p (h w) -> p h w", w=W),
                            scalar=b1s[:, 0:1], in1=vint,
                            op0=Alu.add, op1=Alu.add)
                    s2 = wp.tile([C, PHW], bf16, tag="s2", bufs=3, name=f"s2_{j}")
                    nc.vector.tensor_scalar(
                        out=s2, in0=v2[b], scalar1=1.0, scalar2=None,
                        op0=Alu.is_ge)
                    nc.vector.scalar_tensor_tensor(
                        out=v2[b], in0=v2[b], scalar=1.0, in1=v2[b],
                        op0=Alu.is_lt, op1=Alu.mult)
                    ps2 = conv(s2, w2s, conv2_splits, "ps2")
                    ot = wp.tile([C, HW], f32, tag="ot", bufs=3, name=f"ot_{j}")
                    for h2 in range(2):
                        sl = slice(h2 * (HW // 2), (h2 + 1) * (HW // 2))
                        nc.scalar.activation(
                            out=ot[:, sl], in_=ps2[h2], func=Act.Identity,
                            bias=b2s[:, 0:1], scale=1.0)
                        nc.sync.dma_start(out=y_d[j][:, sl], in_=ot[:, sl])

    nc.finalize()
    return nc


def _split_weights(wf, n_splits):
    # wf: [O, I, 3, 3] float64 (BN scale already folded)
    lhsT = np.transpose(wf, (2, 3, 1, 0)).reshape(9, C, C)  # [tap, ci, co]
    comps, rem = [], lhsT.copy()
    for _ in range(n_splits):
        c = rem.astype(ml_dtypes.bfloat16)
        comps.append(np.asarray(c))
        rem = rem - c.astype(np.float64)
    # [tap, split, ci, co] -> [ci, tap, split, co] -> [ci, tap*split*co]
    a = np.stack(comps, axis=1)
    return np.ascontiguousarray(
        a.transpose(2, 0, 1, 3).reshape(C, 9 * n_splits * C))


def _prep(w, g, b, m, v, n_splits):
    inv = g.astype(np.float64) / np.sqrt(v.astype(np.float64) + EPS)
    wf = w.astype(np.float64) * inv[:, None, None, None]
    shift = (b.astype(np.float64) - m.astype(np.float64) * inv)
    return (_split_weights(wf, n_splits),
            shift.astype(np.float32).reshape(C, 1))


last_results = None  # BassKernelResults of the most recent run (for test.py)


def kernel(x, w1, g1, b1, m1, v1, w2, g2, b2, m2, v2, _trace=False):
    global last_results
    from concourse.bass_utils import run_bass_kernel_spmd

    x = np.asarray(x)
    assert x.shape == (T, B, C, H, W), x.shape

    key = (CONV1_SPLITS, CONV2_SPLITS)
    if key not in _program_cache:
        _program_cache[key] = build_program(
            conv1_splits=CONV1_SPLITS, conv2_splits=CONV2_SPLITS)
    nc = _program_cache[key]

    w1p, sh1 = _prep(np.asarray(w1), np.asarray(g1), np.asarray(b1),
                     np.asarray(m1), np.asarray(v1), CONV1_SPLITS)
    w2p, sh2 = _prep(np.asarray(w2), np.asarray(g2), np.asarray(b2),
                     np.asarray(m2), np.asarray(v2), CONV2_SPLITS)

    in_maps = []
    for c in range(N_CORES):
        xs = np.ascontiguousarray(
            x[:, c * NB:(c + 1) * NB].reshape(N_IMGS, C, HW))
        in_maps.append({"x": xs, "w1": w1p, "w2": w2p, "b1": sh1, "b2": sh2})

    last_results = run_bass_kernel_spmd(
        nc, in_maps, list(range(N_CORES)), trace=_trace)
    res = last_results.results
    out = np.empty((T, B, C, H, W), np.float32)
    for c in range(N_CORES):
        out[:, c * NB:(c + 1) * NB] = res[c]["y"].reshape(T, NB, C, H, W)
    return out
